# revision 6
# baseline (speedup 1.0000x reference)
"""Trainium2 Bass kernel for nn_ComplexCrossAttention.

Sharding: 8 cores = 2 batches x 4 head-groups (4 heads each).
Each core computes, for its (b, head-group):
  - complex Q/K/V projections (column-sharded by head) in transposed layout
  - attention scoresT = (qr.kr + qi.ki)*scale with s on partitions
  - softmax (no max-subtraction; scores are provably small) via exp + column-sum
  - av in transposed layout -> OT [d2, l]
  - partial output projection (row-sharded by head)
Host sums the 4 partial y per batch (bf16 partials) and adds the bias.

All activations are uploaded HOST-TRANSPOSED in bf16 ([C, L] layout), so the
kernel has no cast or DMA-transpose pipeline; weights are uploaded p-major so
every weight DMA row is >= 2KB contiguous. Outputs return as bf16 partials.
"""

import sys

import numpy as np

try:
    import concourse.bacc as bacc
except ImportError:  # pragma: no cover - fallback for bare environments
    sys.path.insert(0, "/opt/trn_rl_repo")
    import concourse.bacc as bacc

import concourse.mybir as mybir
import concourse.tile as tile
from concourse.bass_utils import run_bass_kernel_spmd

F32 = mybir.dt.float32
BF16 = mybir.dt.bfloat16
F32R = mybir.dt.float32r

# ---- problem constants (hardcoded per contract) ----
B, L, S, C = 2, 2048, 2048, 1024
H, D = 16, 64
SCALE = float(1.0 / np.sqrt(np.float32(D)))
HPC = 4          # heads per core
D2 = 2 * D       # stacked (real|imag) head dim = 128
NCK = C // 128   # contraction chunks = 8
NLB = L // 512   # l-blocks = 4
NST = S // 128   # s-tiles = 16
NLT = L // 128   # l-tiles = 16
NEB = 2          # e-blocks of 512 in C

# ---- dtype configuration ----
QS_DT = BF16     # Qs/Ks (scores operands)
EXP_DT = BF16    # expT / Vs / ones (av + denom operands)
VS_DT = EXP_DT
OT_DT = BF16     # OT / wo (o-proj operands)

_CACHE = {}


def _build_program():
    nc = bacc.Bacc("TRN2", target_bir_lowering=False, debug=False, num_devices=8)

    # per-core external inputs: host-transposed bf16 activations [C, L]
    xt_r = nc.dram_tensor("xt_r", [C, L], BF16, kind="ExternalInput")
    xt_i = nc.dram_tensor("xt_i", [C, L], BF16, kind="ExternalInput")
    ct_r = nc.dram_tensor("ct_r", [C, S], BF16, kind="ExternalInput")
    ct_i = nc.dram_tensor("ct_i", [C, S], BF16, kind="ExternalInput")
    # stacked complex projection weights (host-prepared, bf16, p-major)
    # wq/wk: [128, NCK, HPC, 2, D2]  lhsT tiles
    wq = nc.dram_tensor("wq", [128, NCK, HPC, 2, D2], BF16, kind="ExternalInput")
    wk = nc.dram_tensor("wk", [128, NCK, HPC, 2, D2], BF16, kind="ExternalInput")
    # wv: [128, NCK, 2, HPC*128]  rhs tiles
    wv = nc.dram_tensor("wv", [128, NCK, 2, HPC * D2], BF16, kind="ExternalInput")
    # wo: [128, HPC, 2, NEB, 512]  rhs tiles (p = d2 row)
    wo = nc.dram_tensor("wo", [D2, HPC, 2, NEB, 512], OT_DT, kind="ExternalInput")

    y_r = nc.dram_tensor("y_r", [L, C], BF16, kind="ExternalOutput")
    y_i = nc.dram_tensor("y_i", [L, C], BF16, kind="ExternalOutput")

    with tile.TileContext(nc) as tc:
        _emit(nc, tc, xt_r, xt_i, ct_r, ct_i, wq, wk, wv, wo, y_r, y_i)

    nc.compile()
    return nc


def _emit(nc, tc, xt_r, xt_i, ct_r, ct_i, wq, wk, wv, wo, y_r, y_i):
    from contextlib import ExitStack

    ctx = ExitStack()
    with ctx:
        attn_sb = ctx.enter_context(tc.tile_pool(name="attn_sb", bufs=1))

        # persistent attention operands
        qs = attn_sb.tile([128, HPC, L], QS_DT)            # [d2, h, l]
        ks = attn_sb.tile([128, HPC, S], QS_DT)            # [d2, h, s]
        vs = attn_sb.tile([128, NST, HPC * D2], VS_DT)     # [s-part, st, d2all]

        # ---------- P1+P2: load xt + wq, Q projection ----------
        with (
            tc.tile_pool(name="xt", bufs=1) as xt_pool,
            tc.tile_pool(name="wqk", bufs=1) as wqk_pool,
            tc.tile_pool(name="ps_proj", bufs=1, space="PSUM") as ps_proj,
        ):
            wq_sb = wqk_pool.tile([128, NCK, HPC, 2, D2], BF16, tag="wq")
            xt = [xt_pool.tile([128, NCK, L], BF16, tag=f"xt{t}", name=f"xt{t}") for t in range(2)]
            for ck in range(NCK):
                csl = slice(ck * 128, (ck + 1) * 128)
                nc.gpsimd.dma_start(out=wq_sb[:, ck], in_=wq[:, ck])
                nc.sync.dma_start(out=xt[0][:, ck, :], in_=xt_r[csl, :])
                nc.scalar.dma_start(out=xt[1][:, ck, :], in_=xt_i[csl, :])
            for hp in range(HPC // 2):
                pq = [
                    [ps_proj.tile([128, 512], F32, tag=f"pq{hh}{lb}", name=f"pq{hh}{lb}")
                     for lb in range(NLB)]
                    for hh in range(2)
                ]
                n = 2 * NCK
                i = 0
                for ck in range(NCK):
                    for pm in range(2):
                        for hh in range(2):
                            for lb in range(NLB):
                                nc.tensor.matmul(
                                    pq[hh][lb],
                                    wq_sb[:, ck, 2 * hp + hh, pm, :],
                                    xt[pm][:, ck, lb * 512:(lb + 1) * 512],
                                    start=(i == 0),
                                    stop=(i == n - 1),
                                )
                        i += 1
                for hh in range(2):
                    for lb in range(NLB):
                        nc.vector.tensor_copy(
                            out=qs[:, 2 * hp + hh, lb * 512:(lb + 1) * 512], in_=pq[hh][lb]
                        )

        # ---------- P3: load ct + wk/wv, K and V projections ----------
        exp_pool = ctx.enter_context(tc.tile_pool(name="exp", bufs=2))
        ps_s = ctx.enter_context(tc.tile_pool(name="ps_s", bufs=2, space="PSUM"))

        with (
            tc.tile_pool(name="ct", bufs=1) as ct_pool,
            tc.tile_pool(name="wkv", bufs=1) as wkv_pool,
            tc.tile_pool(name="ps_proj2", bufs=1, space="PSUM") as ps_proj,
            tc.tile_pool(name="ps_v", bufs=2, space="PSUM") as ps_v,
        ):
            wk_sb = wkv_pool.tile([128, NCK, HPC, 2, D2], BF16, tag="wkv")
            wv_sb = wkv_pool.tile([128, NCK, 2, HPC * D2], BF16, tag="wkv", name="wv_sb")
            ct = [ct_pool.tile([128, NCK, S], BF16, tag=f"ct{t}", name=f"ct{t}") for t in range(2)]
            for ck in range(NCK):
                csl = slice(ck * 128, (ck + 1) * 128)
                nc.gpsimd.dma_start(out=wk_sb[:, ck], in_=wk[:, ck])
                nc.sync.dma_start(out=ct[0][:, ck, :], in_=ct_r[csl, :])
                nc.scalar.dma_start(out=ct[1][:, ck, :], in_=ct_i[csl, :])
                nc.gpsimd.dma_start(out=wv_sb[:, ck], in_=wv[:, ck])
            for h in range(HPC):
                for rnd in range(2):
                    pk = [ps_proj.tile([128, 512], F32, tag=f"pk{j}", name=f"pk{j}") for j in range(2)]
                    n = 2 * NCK
                    i = 0
                    for ck in range(NCK):
                        for pm in range(2):
                            for j in range(2):
                                sb = 2 * rnd + j
                                nc.tensor.matmul(
                                    pk[j],
                                    wk_sb[:, ck, h, pm, :],
                                    ct[pm][:, ck, sb * 512:(sb + 1) * 512],
                                    start=(i == 0),
                                    stop=(i == n - 1),
                                )
                            i += 1
                    for j in range(2):
                        sb = 2 * rnd + j
                        nc.vector.tensor_copy(out=ks[:, h, sb * 512:(sb + 1) * 512], in_=pk[j])
            for st in range(NST):
                pv = ps_v.tile([128, 512], F32, tag="pv")
                n = 2 * NCK
                i = 0
                for ck in range(NCK):
                    for pm in range(2):
                        nc.tensor.matmul(
                            pv,
                            ct[pm][:, ck, st * 128:(st + 1) * 128],
                            wv_sb[:, ck, pm, :],
                            start=(i == 0),
                            stop=(i == n - 1),
                        )
                        i += 1
                nc.vector.tensor_copy(out=vs[:, st, :], in_=pv)

        # ---------- P4+P5 fused: attention + output projection, lb-outer ----------
        with (
            tc.tile_pool(name="late", bufs=1) as late_pool,
            tc.tile_pool(name="exp2", bufs=3) as exp_pool2,
            tc.tile_pool(name="otp", bufs=2) as ot_pool,
            tc.tile_pool(name="ysb", bufs=4) as ysb_pool,
            tc.tile_pool(name="ps_d", bufs=1, space="PSUM") as ps_d,
            tc.tile_pool(name="ps_o", bufs=1, space="PSUM") as ps_o,
            tc.tile_pool(name="ps_y", bufs=1, space="PSUM") as ps_y,
        ):
            ones = late_pool.tile([128, D2], EXP_DT)
            nc.vector.memset(ones, 1.0)
            wo_sb = late_pool.tile([128, HPC, 2, NEB, 512], OT_DT, tag="wo", name="wo_sb")
            nc.gpsimd.dma_start(out=wo_sb, in_=wo[:, :, :, :, :])
            for lb in range(NLB):
                lsl = slice(lb * 512, (lb + 1) * 512)
                ot = ot_pool.tile([128, HPC, 512], OT_DT, tag="ot", name="ot")
                for h in range(HPC):
                    pool_h = exp_pool if lb == 0 else exp_pool2
                    expt = pool_h.tile([128, NST, 512], EXP_DT, tag="expt", name="expt")
                    for pr in range(NST // 2):
                        pscore = ps_s.tile([128, 2, 512], F32, tag="pscore")
                        for j in range(2):
                            st = 2 * pr + j
                            nc.tensor.matmul(
                                pscore[:, j, :],
                                ks[:, h, st * 128:(st + 1) * 128],
                                qs[:, h, lsl],
                                start=True,
                                stop=True,
                                skip_group_check=True,
                            )
                        nc.scalar.activation(
                            out=expt[:, 2 * pr:2 * pr + 2, :],
                            in_=pscore,
                            func=mybir.ActivationFunctionType.Exp,
                            scale=SCALE,
                        )
                    # av: OT[d2, l] accumulated over s-tiles (reads expt first)
                    pav = ps_o.tile([128, 512], F32, tag="pav")
                    for st in range(NST):
                        nc.tensor.matmul(
                            pav,
                            vs[:, st, h * D2:(h + 1) * D2],
                            expt[:, st, :],
                            start=(st == 0),
                            stop=(st == NST - 1),
                        )
                    # out-of-place tree-sum of the 16 s-tiles: runs parallel
                    # with av (expt is only read), strided pairwise levels
                    tr1 = ot_pool.tile([128, 8, 512], EXP_DT, tag="tr1")
                    tr2 = ot_pool.tile([128, 4, 512], EXP_DT, tag="tr2")
                    ev = expt.rearrange("p (a two) f -> p a two f", two=2)
                    nc.gpsimd.tensor_add(out=tr1, in0=ev[:, :, 0, :], in1=ev[:, :, 1, :])
                    t1v = tr1.rearrange("p (a two) f -> p a two f", two=2)
                    nc.vector.tensor_add(
                        out=tr2, in0=t1v[:, :, 0, :], in1=t1v[:, :, 1, :])
                    t2v = tr2.rearrange("p (a two) f -> p a two f", two=2)
                    nc.vector.tensor_add(
                        out=tr1[:, 0:2, :], in0=t2v[:, :, 0, :], in1=t2v[:, :, 1, :])
                    nc.vector.tensor_add(
                        out=tr2[:, 0, :], in0=tr1[:, 0, :], in1=tr1[:, 1, :])
                    pden = ps_d.tile([128, 512], F32, tag="pden")
                    nc.tensor.matmul(pden, ones, tr2[:, 0, :], start=True, stop=True)
                    recip = ot_pool.tile([128, 512], F32, tag="recip")
                    nc.vector.reciprocal(out=recip, in_=pden)
                    nc.vector.tensor_mul(out=ot[:, h, :], in0=pav, in1=recip)

                # output projection for this l-block (needs all heads' ot)
                for jt in range(4):
                    lt = lb * 4 + jt
                    lrow = slice(lt * 128, (lt + 1) * 128)
                    for eb in range(NEB):
                        esl = slice(eb * 512, (eb + 1) * 512)
                        py = [ps_y.tile([128, 512], F32, tag=f"py{ri}", name=f"py{ri}")
                              for ri in range(2)]
                        for h in range(HPC):
                            for ri in range(2):
                                nc.tensor.matmul(
                                    py[ri],
                                    ot[:, h, jt * 128:(jt + 1) * 128],
                                    wo_sb[:, h, ri, eb, :],
                                    start=(h == 0),
                                    stop=(h == HPC - 1),
                                )
                        yr_t = ysb_pool.tile([128, 512], BF16, tag="yrt")
                        nc.vector.tensor_copy(out=yr_t, in_=py[0])
                        nc.sync.dma_start(out=y_r[lrow, esl], in_=yr_t)
                        yi_t = ysb_pool.tile([128, 512], BF16, tag="yit")
                        nc.vector.tensor_copy(out=yi_t, in_=py[1])
                        nc.sync.dma_start(out=y_i[lrow, esl], in_=yi_t)


def _prep_core_inputs(inputs, core):
    """Slice + host-prepare activation/weight layouts for one core."""
    import ml_dtypes

    BFD = ml_dtypes.bfloat16
    b = core // 4
    g = core % 4
    hcols = slice(g * HPC * D, (g + 1) * HPC * D)  # 256 channel cols/rows

    wq_r = inputs["wq_r"][:, hcols]
    wq_i = inputs["wq_i"][:, hcols]
    wk_r = inputs["wk_r"][:, hcols]
    wk_i = inputs["wk_i"][:, hcols]
    wv_r = inputs["wv_r"][:, hcols]
    wv_i = inputs["wv_i"][:, hcols]
    wo_r = inputs["wo_r"][hcols, :]
    wo_i = inputs["wo_i"][hcols, :]

    def pmajor(w):
        # [C, ...] -> [128, NCK, ...]
        return np.ascontiguousarray(
            w.reshape(NCK, 128, *w.shape[1:]).transpose(
                1, 0, *range(2, w.ndim + 1))
        )

    def stack_lhst(wr, wi):
        # [C, HPC, 2, D2]: pm=0 -> [wr | wi], pm=1 -> [-wi | wr]
        out = np.empty((C, HPC, 2, D2), np.float32)
        for hh in range(HPC):
            cs = slice(hh * D, (hh + 1) * D)
            out[:, hh, 0, :D] = wr[:, cs]
            out[:, hh, 0, D:] = wi[:, cs]
            out[:, hh, 1, :D] = -wi[:, cs]
            out[:, hh, 1, D:] = wr[:, cs]
        return pmajor(out.astype(BFD))

    def stack_rhs_v(wr, wi):
        # [C, 2, HPC*D2]
        out = np.empty((C, 2, HPC * D2), np.float32)
        for hh in range(HPC):
            cs = slice(hh * D, (hh + 1) * D)
            out[:, 0, hh * D2:hh * D2 + D] = wr[:, cs]
            out[:, 0, hh * D2 + D:(hh + 1) * D2] = wi[:, cs]
            out[:, 1, hh * D2:hh * D2 + D] = -wi[:, cs]
            out[:, 1, hh * D2 + D:(hh + 1) * D2] = wr[:, cs]
        return pmajor(out.astype(BFD))

    def stack_wo(wr, wi):
        # [D2, HPC, 2, NEB, 512]; rows 0:D multiply Or, D:D2 multiply Oi
        out = np.empty((D2, HPC, 2, NEB, 512), np.float32)
        for hh in range(HPC):
            rs = slice(hh * D, (hh + 1) * D)
            for eb in range(NEB):
                esl = slice(eb * 512, (eb + 1) * 512)
                out[:D, hh, 0, eb, :] = wr[rs, esl]
                out[D:, hh, 0, eb, :] = -wi[rs, esl]
                out[:D, hh, 1, eb, :] = wi[rs, esl]
                out[D:, hh, 1, eb, :] = wr[rs, esl]
        return np.ascontiguousarray(out.astype(BFD))

    def tcast(x):
        return np.ascontiguousarray(x.T.astype(BFD))

    return {
        "xt_r": tcast(inputs["inputs_real"][b]),
        "xt_i": tcast(inputs["inputs_imag"][b]),
        "ct_r": tcast(inputs["context_real"][b]),
        "ct_i": tcast(inputs["context_imag"][b]),
        "wq": stack_lhst(wq_r, wq_i),
        "wk": stack_lhst(wk_r, wk_i),
        "wv": stack_rhs_v(wv_r, wv_i),
        "wo": stack_wo(wo_r, wo_i),
    }


def get_program():
    if "nc" not in _CACHE:
        _CACHE["nc"] = _build_program()
    return _CACHE["nc"]


def kernel(**inputs):
    nc = get_program()
    in_maps = [_prep_core_inputs(inputs, core) for core in range(8)]
    res = run_bass_kernel_spmd(nc, in_maps, core_ids=list(range(8)))

    yr = np.zeros((B, L, C), np.float32)
    yi = np.zeros((B, L, C), np.float32)
    for core in range(8):
        b = core // 4
        yr[b] += res.results[core]["y_r"].astype(np.float32)
        yi[b] += res.results[core]["y_i"].astype(np.float32)
    yr += inputs["bo_r"][None, None, :]
    yi += inputs["bo_i"][None, None, :]
    return np.stack([yr, yi], axis=0)


# revision 7
# speedup vs baseline: 1.0463x; 1.0463x over previous
"""Trainium2 Bass kernel for nn_ComplexCrossAttention.

Sharding: 8 cores = 2 batches x 4 head-groups (4 heads each).
Each core computes, for its (b, head-group):
  - complex Q/K/V projections (column-sharded by head) in transposed layout
  - attention scoresT = (qr.kr + qi.ki)*scale with s on partitions
  - softmax (no max-subtraction; scores are provably small) via exp + column-sum
  - av in transposed layout -> OT [d2, l]
  - partial output projection (row-sharded by head)
Host sums the 4 partial y per batch (bf16 partials) and adds the bias.

All activations are uploaded HOST-TRANSPOSED in bf16 ([C, L] layout), so the
kernel has no cast or DMA-transpose pipeline; weights are uploaded p-major so
every weight DMA row is >= 2KB contiguous. Outputs return as bf16 partials.
"""

import sys

import numpy as np

try:
    import concourse.bacc as bacc
except ImportError:  # pragma: no cover - fallback for bare environments
    sys.path.insert(0, "/opt/trn_rl_repo")
    import concourse.bacc as bacc

import concourse.mybir as mybir
import concourse.tile as tile
from concourse.bass_utils import run_bass_kernel_spmd

F32 = mybir.dt.float32
BF16 = mybir.dt.bfloat16
F32R = mybir.dt.float32r

# ---- problem constants (hardcoded per contract) ----
B, L, S, C = 2, 2048, 2048, 1024
H, D = 16, 64
SCALE = float(1.0 / np.sqrt(np.float32(D)))
HPC = 4          # heads per core
D2 = 2 * D       # stacked (real|imag) head dim = 128
NCK = C // 128   # contraction chunks = 8
NLB = L // 512   # l-blocks = 4
NST = S // 128   # s-tiles = 16
NLT = L // 128   # l-tiles = 16
NEB = 2          # e-blocks of 512 in C

# ---- dtype configuration ----
QS_DT = BF16     # Qs/Ks (scores operands)
EXP_DT = BF16    # expT / Vs / ones (av + denom operands)
VS_DT = EXP_DT
OT_DT = BF16     # OT / wo (o-proj operands)

_CACHE = {}


def _build_program():
    nc = bacc.Bacc("TRN2", target_bir_lowering=False, debug=False, num_devices=8)

    # per-core external inputs: host-transposed bf16 activations [C, L]
    xt_r = nc.dram_tensor("xt_r", [C, L], BF16, kind="ExternalInput")
    xt_i = nc.dram_tensor("xt_i", [C, L], BF16, kind="ExternalInput")
    ct_r = nc.dram_tensor("ct_r", [C, S], BF16, kind="ExternalInput")
    ct_i = nc.dram_tensor("ct_i", [C, S], BF16, kind="ExternalInput")
    # stacked complex projection weights (host-prepared, bf16, p-major)
    # wq/wk: [128, NCK, HPC, 2, D2]  lhsT tiles
    wq = nc.dram_tensor("wq", [128, NCK, HPC, 2, D2], BF16, kind="ExternalInput")
    wk = nc.dram_tensor("wk", [128, NCK, HPC, 2, D2], BF16, kind="ExternalInput")
    # wv: [128, NCK, 2, HPC*128]  rhs tiles
    wv = nc.dram_tensor("wv", [128, NCK, 2, HPC * D2], BF16, kind="ExternalInput")
    # wo: [128, HPC, 2, NEB, 512]  rhs tiles (p = d2 row)
    wo = nc.dram_tensor("wo", [D2, HPC, 2, NEB, 512], OT_DT, kind="ExternalInput")

    y_r = nc.dram_tensor("y_r", [L, C], BF16, kind="ExternalOutput")
    y_i = nc.dram_tensor("y_i", [L, C], BF16, kind="ExternalOutput")

    with tile.TileContext(nc) as tc:
        _emit(nc, tc, xt_r, xt_i, ct_r, ct_i, wq, wk, wv, wo, y_r, y_i)

    nc.compile()
    return nc


def _emit(nc, tc, xt_r, xt_i, ct_r, ct_i, wq, wk, wv, wo, y_r, y_i):
    from contextlib import ExitStack

    ctx = ExitStack()
    with ctx:
        attn_sb = ctx.enter_context(tc.tile_pool(name="attn_sb", bufs=1))

        # persistent attention operands
        qs = attn_sb.tile([128, HPC, L], QS_DT)            # [d2, h, l]
        ks = attn_sb.tile([128, HPC, S], QS_DT)            # [d2, h, s]
        vs = attn_sb.tile([128, NST, HPC * D2], VS_DT)     # [s-part, st, d2all]

        # ---------- P1+P2: load xt + wq, Q projection ----------
        with (
            tc.tile_pool(name="xt", bufs=1) as xt_pool,
            tc.tile_pool(name="wqk", bufs=1) as wqk_pool,
            tc.tile_pool(name="ps_proj", bufs=1, space="PSUM") as ps_proj,
        ):
            wq_sb = wqk_pool.tile([128, NCK, HPC, 2, D2], BF16, tag="wq")
            xt = [xt_pool.tile([128, NCK, L], BF16, tag=f"xt{t}", name=f"xt{t}") for t in range(2)]
            for ck in range(NCK):
                csl = slice(ck * 128, (ck + 1) * 128)
                nc.gpsimd.dma_start(out=wq_sb[:, ck], in_=wq[:, ck])
                nc.sync.dma_start(out=xt[0][:, ck, :], in_=xt_r[csl, :])
                nc.scalar.dma_start(out=xt[1][:, ck, :], in_=xt_i[csl, :])
            for hp in range(HPC // 2):
                pq = [
                    [ps_proj.tile([128, 512], F32, tag=f"pq{hh}{lb}", name=f"pq{hh}{lb}")
                     for lb in range(NLB)]
                    for hh in range(2)
                ]
                n = 2 * NCK
                i = 0
                for ck in range(NCK):
                    for pm in range(2):
                        for hh in range(2):
                            for lb in range(NLB):
                                nc.tensor.matmul(
                                    pq[hh][lb],
                                    wq_sb[:, ck, 2 * hp + hh, pm, :],
                                    xt[pm][:, ck, lb * 512:(lb + 1) * 512],
                                    start=(i == 0),
                                    stop=(i == n - 1),
                                )
                        i += 1
                for hh in range(2):
                    for lb in range(NLB):
                        nc.vector.tensor_copy(
                            out=qs[:, 2 * hp + hh, lb * 512:(lb + 1) * 512], in_=pq[hh][lb]
                        )

        # ---------- P3: load ct + wk/wv, K and V projections ----------
        exp_pool = ctx.enter_context(tc.tile_pool(name="exp", bufs=2))
        ps_s = ctx.enter_context(tc.tile_pool(name="ps_s", bufs=2, space="PSUM"))

        with (
            tc.tile_pool(name="ct", bufs=1) as ct_pool,
            tc.tile_pool(name="wkv", bufs=1) as wkv_pool,
            tc.tile_pool(name="ps_proj2", bufs=1, space="PSUM") as ps_proj,
            tc.tile_pool(name="ps_v", bufs=2, space="PSUM") as ps_v,
        ):
            wk_sb = wkv_pool.tile([128, NCK, HPC, 2, D2], BF16, tag="wkv")
            wv_sb = wkv_pool.tile([128, NCK, 2, HPC * D2], BF16, tag="wkv", name="wv_sb")
            ct = [ct_pool.tile([128, NCK, S], BF16, tag=f"ct{t}", name=f"ct{t}") for t in range(2)]
            for ck in range(NCK):
                csl = slice(ck * 128, (ck + 1) * 128)
                nc.gpsimd.dma_start(out=wk_sb[:, ck], in_=wk[:, ck])
                nc.sync.dma_start(out=ct[0][:, ck, :], in_=ct_r[csl, :])
                nc.scalar.dma_start(out=ct[1][:, ck, :], in_=ct_i[csl, :])
                nc.gpsimd.dma_start(out=wv_sb[:, ck], in_=wv[:, ck])
            for h in range(HPC):
                for rnd in range(2):
                    pk = [ps_proj.tile([128, 512], F32, tag=f"pk{j}", name=f"pk{j}") for j in range(2)]
                    n = 2 * NCK
                    i = 0
                    for ck in range(NCK):
                        for pm in range(2):
                            for j in range(2):
                                sb = 2 * rnd + j
                                nc.tensor.matmul(
                                    pk[j],
                                    wk_sb[:, ck, h, pm, :],
                                    ct[pm][:, ck, sb * 512:(sb + 1) * 512],
                                    start=(i == 0),
                                    stop=(i == n - 1),
                                )
                            i += 1
                    for j in range(2):
                        sb = 2 * rnd + j
                        nc.vector.tensor_copy(out=ks[:, h, sb * 512:(sb + 1) * 512], in_=pk[j])
            for st in range(NST):
                pv = ps_v.tile([128, 512], F32, tag="pv")
                n = 2 * NCK
                i = 0
                for ck in range(NCK):
                    for pm in range(2):
                        nc.tensor.matmul(
                            pv,
                            ct[pm][:, ck, st * 128:(st + 1) * 128],
                            wv_sb[:, ck, pm, :],
                            start=(i == 0),
                            stop=(i == n - 1),
                        )
                        i += 1
                nc.vector.tensor_copy(out=vs[:, st, :], in_=pv)

        # ---------- P4+P5 fused: attention + output projection, lb-outer ----------
        with (
            tc.tile_pool(name="late", bufs=1) as late_pool,
            tc.tile_pool(name="exp2", bufs=3) as exp_pool2,
            tc.tile_pool(name="otp", bufs=2) as ot_pool,
            tc.tile_pool(name="ysb", bufs=4) as ysb_pool,
            tc.tile_pool(name="ps_d", bufs=1, space="PSUM") as ps_d,
            tc.tile_pool(name="ps_o", bufs=1, space="PSUM") as ps_o,
            tc.tile_pool(name="ps_y", bufs=1, space="PSUM") as ps_y,
        ):
            ones = late_pool.tile([128, D2], EXP_DT)
            nc.vector.memset(ones, 1.0)
            wo_sb = late_pool.tile([128, HPC, 2, NEB, 512], OT_DT, tag="wo", name="wo_sb")
            nc.gpsimd.dma_start(out=wo_sb, in_=wo[:, :, :, :, :])
            for lb in range(NLB):
                lsl = slice(lb * 512, (lb + 1) * 512)
                ot = ot_pool.tile([128, HPC, 512], OT_DT, tag="ot", name="ot")
                for h in range(HPC):
                    pool_h = exp_pool if lb == 0 else exp_pool2
                    expt = pool_h.tile([128, NST, 512], EXP_DT, tag="expt", name="expt")
                    for pr in range(NST // 2):
                        pscore = ps_s.tile([128, 2, 512], F32, tag="pscore")
                        for j in range(2):
                            st = 2 * pr + j
                            nc.tensor.matmul(
                                pscore[:, j, :],
                                ks[:, h, st * 128:(st + 1) * 128],
                                qs[:, h, lsl],
                                start=True,
                                stop=True,
                                skip_group_check=True,
                            )
                        nc.scalar.activation(
                            out=expt[:, 2 * pr:2 * pr + 2, :],
                            in_=pscore,
                            func=mybir.ActivationFunctionType.Exp,
                            scale=SCALE,
                        )
                    # av: OT[d2, l] accumulated over s-tiles (reads expt first)
                    pav = ps_o.tile([128, 512], F32, tag="pav")
                    for st in range(NST):
                        nc.tensor.matmul(
                            pav,
                            vs[:, st, h * D2:(h + 1) * D2],
                            expt[:, st, :],
                            start=(st == 0),
                            stop=(st == NST - 1),
                        )
                    # out-of-place tree-sum of the 16 s-tiles: runs parallel
                    # with av (expt is only read), strided pairwise levels
                    tr1 = ot_pool.tile([128, 8, 512], EXP_DT, tag="tr1")
                    tr2 = ot_pool.tile([128, 4, 512], EXP_DT, tag="tr2")
                    ev = expt.rearrange("p (a two) f -> p a two f", two=2)
                    for q in range(4):
                        eng = nc.gpsimd if q % 2 == 0 else nc.vector
                        eng.tensor_add(
                            out=tr1[:, 2 * q:2 * q + 2, :],
                            in0=ev[:, 2 * q:2 * q + 2, 0, :],
                            in1=ev[:, 2 * q:2 * q + 2, 1, :],
                        )
                    t1v = tr1.rearrange("p (a two) f -> p a two f", two=2)
                    for q in range(2):
                        nc.vector.tensor_add(
                            out=tr2[:, 2 * q:2 * q + 2, :],
                            in0=t1v[:, 2 * q:2 * q + 2, 0, :],
                            in1=t1v[:, 2 * q:2 * q + 2, 1, :],
                        )
                    t2v = tr2.rearrange("p (a two) f -> p a two f", two=2)
                    nc.vector.tensor_add(
                        out=tr1[:, 0:2, :], in0=t2v[:, :, 0, :], in1=t2v[:, :, 1, :])
                    nc.vector.tensor_add(
                        out=tr2[:, 0, :], in0=tr1[:, 0, :], in1=tr1[:, 1, :])
                    pden = ps_d.tile([128, 512], F32, tag="pden")
                    nc.tensor.matmul(pden, ones, tr2[:, 0, :], start=True, stop=True)
                    recip = ot_pool.tile([128, 512], F32, tag="recip")
                    nc.vector.reciprocal(out=recip, in_=pden)
                    nc.vector.tensor_mul(out=ot[:, h, :], in0=pav, in1=recip)

                # output projection for this l-block (needs all heads' ot)
                for jt in range(4):
                    lt = lb * 4 + jt
                    lrow = slice(lt * 128, (lt + 1) * 128)
                    for eb in range(NEB):
                        esl = slice(eb * 512, (eb + 1) * 512)
                        py = [ps_y.tile([128, 512], F32, tag=f"py{ri}", name=f"py{ri}")
                              for ri in range(2)]
                        for h in range(HPC):
                            for ri in range(2):
                                nc.tensor.matmul(
                                    py[ri],
                                    ot[:, h, jt * 128:(jt + 1) * 128],
                                    wo_sb[:, h, ri, eb, :],
                                    start=(h == 0),
                                    stop=(h == HPC - 1),
                                )
                        yr_t = ysb_pool.tile([128, 512], BF16, tag="yrt")
                        nc.vector.tensor_copy(out=yr_t, in_=py[0])
                        nc.sync.dma_start(out=y_r[lrow, esl], in_=yr_t)
                        yi_t = ysb_pool.tile([128, 512], BF16, tag="yit")
                        nc.vector.tensor_copy(out=yi_t, in_=py[1])
                        nc.sync.dma_start(out=y_i[lrow, esl], in_=yi_t)


def _prep_core_inputs(inputs, core):
    """Slice + host-prepare activation/weight layouts for one core."""
    import ml_dtypes

    BFD = ml_dtypes.bfloat16
    b = core // 4
    g = core % 4
    hcols = slice(g * HPC * D, (g + 1) * HPC * D)  # 256 channel cols/rows

    wq_r = inputs["wq_r"][:, hcols]
    wq_i = inputs["wq_i"][:, hcols]
    wk_r = inputs["wk_r"][:, hcols]
    wk_i = inputs["wk_i"][:, hcols]
    wv_r = inputs["wv_r"][:, hcols]
    wv_i = inputs["wv_i"][:, hcols]
    wo_r = inputs["wo_r"][hcols, :]
    wo_i = inputs["wo_i"][hcols, :]

    def pmajor(w):
        # [C, ...] -> [128, NCK, ...]
        return np.ascontiguousarray(
            w.reshape(NCK, 128, *w.shape[1:]).transpose(
                1, 0, *range(2, w.ndim + 1))
        )

    def stack_lhst(wr, wi):
        # [C, HPC, 2, D2]: pm=0 -> [wr | wi], pm=1 -> [-wi | wr]
        out = np.empty((C, HPC, 2, D2), np.float32)
        for hh in range(HPC):
            cs = slice(hh * D, (hh + 1) * D)
            out[:, hh, 0, :D] = wr[:, cs]
            out[:, hh, 0, D:] = wi[:, cs]
            out[:, hh, 1, :D] = -wi[:, cs]
            out[:, hh, 1, D:] = wr[:, cs]
        return pmajor(out.astype(BFD))

    def stack_rhs_v(wr, wi):
        # [C, 2, HPC*D2]
        out = np.empty((C, 2, HPC * D2), np.float32)
        for hh in range(HPC):
            cs = slice(hh * D, (hh + 1) * D)
            out[:, 0, hh * D2:hh * D2 + D] = wr[:, cs]
            out[:, 0, hh * D2 + D:(hh + 1) * D2] = wi[:, cs]
            out[:, 1, hh * D2:hh * D2 + D] = -wi[:, cs]
            out[:, 1, hh * D2 + D:(hh + 1) * D2] = wr[:, cs]
        return pmajor(out.astype(BFD))

    def stack_wo(wr, wi):
        # [D2, HPC, 2, NEB, 512]; rows 0:D multiply Or, D:D2 multiply Oi
        out = np.empty((D2, HPC, 2, NEB, 512), np.float32)
        for hh in range(HPC):
            rs = slice(hh * D, (hh + 1) * D)
            for eb in range(NEB):
                esl = slice(eb * 512, (eb + 1) * 512)
                out[:D, hh, 0, eb, :] = wr[rs, esl]
                out[D:, hh, 0, eb, :] = -wi[rs, esl]
                out[:D, hh, 1, eb, :] = wi[rs, esl]
                out[D:, hh, 1, eb, :] = wr[rs, esl]
        return np.ascontiguousarray(out.astype(BFD))

    def tcast(x):
        return np.ascontiguousarray(x.T.astype(BFD))

    return {
        "xt_r": tcast(inputs["inputs_real"][b]),
        "xt_i": tcast(inputs["inputs_imag"][b]),
        "ct_r": tcast(inputs["context_real"][b]),
        "ct_i": tcast(inputs["context_imag"][b]),
        "wq": stack_lhst(wq_r, wq_i),
        "wk": stack_lhst(wk_r, wk_i),
        "wv": stack_rhs_v(wv_r, wv_i),
        "wo": stack_wo(wo_r, wo_i),
    }


def get_program():
    if "nc" not in _CACHE:
        _CACHE["nc"] = _build_program()
    return _CACHE["nc"]


def kernel(**inputs):
    nc = get_program()
    in_maps = [_prep_core_inputs(inputs, core) for core in range(8)]
    res = run_bass_kernel_spmd(nc, in_maps, core_ids=list(range(8)))

    yr = np.zeros((B, L, C), np.float32)
    yi = np.zeros((B, L, C), np.float32)
    for core in range(8):
        b = core // 4
        yr[b] += res.results[core]["y_r"].astype(np.float32)
        yi[b] += res.results[core]["y_i"].astype(np.float32)
    yr += inputs["bo_r"][None, None, :]
    yi += inputs["bo_i"][None, None, :]
    return np.stack([yr, yi], axis=0)


# revision 9
# speedup vs baseline: 1.0480x; 1.0016x over previous
"""Trainium2 Bass kernel for nn_ComplexCrossAttention.

Sharding: 8 cores = 2 batches x 4 head-groups (4 heads each).
Each core computes, for its (b, head-group):
  - complex Q/K/V projections (column-sharded by head) in transposed layout
  - attention scoresT = (qr.kr + qi.ki)*scale with s on partitions
  - softmax (no max-subtraction; scores are provably small) via exp + column-sum
  - av in transposed layout -> OT [d2, l]
  - partial output projection (row-sharded by head)
Host sums the 4 partial y per batch (bf16 partials) and adds the bias.

All activations are uploaded HOST-TRANSPOSED in bf16 ([C, L] layout), so the
kernel has no cast or DMA-transpose pipeline; weights are uploaded p-major so
every weight DMA row is >= 2KB contiguous. Outputs return as bf16 partials.
"""

import sys

import numpy as np

try:
    import concourse.bacc as bacc
except ImportError:  # pragma: no cover - fallback for bare environments
    sys.path.insert(0, "/opt/trn_rl_repo")
    import concourse.bacc as bacc

import concourse.mybir as mybir
import concourse.tile as tile
from concourse.bass_utils import run_bass_kernel_spmd

F32 = mybir.dt.float32
BF16 = mybir.dt.bfloat16
F32R = mybir.dt.float32r

# ---- problem constants (hardcoded per contract) ----
B, L, S, C = 2, 2048, 2048, 1024
H, D = 16, 64
SCALE = float(1.0 / np.sqrt(np.float32(D)))
HPC = 4          # heads per core
D2 = 2 * D       # stacked (real|imag) head dim = 128
NCK = C // 128   # contraction chunks = 8
NLB = L // 512   # l-blocks = 4
NST = S // 128   # s-tiles = 16
NLT = L // 128   # l-tiles = 16
NEB = 2          # e-blocks of 512 in C

# ---- dtype configuration ----
QS_DT = BF16     # Qs/Ks (scores operands)
EXP_DT = BF16    # expT / Vs / ones (av + denom operands)
VS_DT = EXP_DT
OT_DT = BF16     # OT / wo (o-proj operands)

_CACHE = {}


def _build_program():
    nc = bacc.Bacc("TRN2", target_bir_lowering=False, debug=False, num_devices=8)

    # per-core external inputs: host-transposed bf16 activations [C, L]
    xt_r = nc.dram_tensor("xt_r", [C, L], BF16, kind="ExternalInput")
    xt_i = nc.dram_tensor("xt_i", [C, L], BF16, kind="ExternalInput")
    ct_r = nc.dram_tensor("ct_r", [C, S], BF16, kind="ExternalInput")
    ct_i = nc.dram_tensor("ct_i", [C, S], BF16, kind="ExternalInput")
    # stacked complex projection weights (host-prepared, bf16, p-major)
    # wq/wk: [128, NCK, HPC, 2, D2]  lhsT tiles
    wq = nc.dram_tensor("wq", [128, NCK, HPC, 2, D2], BF16, kind="ExternalInput")
    wk = nc.dram_tensor("wk", [128, NCK, HPC, 2, D2], BF16, kind="ExternalInput")
    # wv: [128, NCK, 2, HPC*128]  rhs tiles
    wv = nc.dram_tensor("wv", [128, NCK, 2, HPC * D2], BF16, kind="ExternalInput")
    # wo: [128, HPC, 2, NEB, 512]  rhs tiles (p = d2 row)
    wo = nc.dram_tensor("wo", [D2, HPC, 2, NEB, 512], OT_DT, kind="ExternalInput")

    y_r = nc.dram_tensor("y_r", [L, C], BF16, kind="ExternalOutput")
    y_i = nc.dram_tensor("y_i", [L, C], BF16, kind="ExternalOutput")

    with tile.TileContext(nc) as tc:
        _emit(nc, tc, xt_r, xt_i, ct_r, ct_i, wq, wk, wv, wo, y_r, y_i)

    nc.compile()
    return nc


def _emit(nc, tc, xt_r, xt_i, ct_r, ct_i, wq, wk, wv, wo, y_r, y_i):
    from contextlib import ExitStack

    ctx = ExitStack()
    with ctx:
        attn_sb = ctx.enter_context(tc.tile_pool(name="attn_sb", bufs=1))

        # persistent attention operands
        qs = attn_sb.tile([128, HPC, L], QS_DT)            # [d2, h, l]
        ks = attn_sb.tile([128, HPC, S], QS_DT)            # [d2, h, s]
        vs = attn_sb.tile([128, NST, HPC * D2], VS_DT)     # [s-part, st, d2all]

        # ---------- P1+P2: load xt + wq, Q projection ----------
        with (
            tc.tile_pool(name="xt", bufs=1) as xt_pool,
            tc.tile_pool(name="wqk", bufs=1) as wqk_pool,
            tc.tile_pool(name="ps_proj", bufs=1, space="PSUM") as ps_proj,
        ):
            wq_sb = wqk_pool.tile([128, NCK, HPC, 2, D2], BF16, tag="wq")
            xt = [xt_pool.tile([128, NCK, L], BF16, tag=f"xt{t}", name=f"xt{t}") for t in range(2)]
            for ck in range(NCK):
                csl = slice(ck * 128, (ck + 1) * 128)
                nc.gpsimd.dma_start(out=wq_sb[:, ck], in_=wq[:, ck])
                nc.sync.dma_start(out=xt[0][:, ck, :], in_=xt_r[csl, :])
                nc.scalar.dma_start(out=xt[1][:, ck, :], in_=xt_i[csl, :])
            for hp in range(HPC // 2):
                pq = [
                    [ps_proj.tile([128, 512], F32, tag=f"pq{hh}{lb}", name=f"pq{hh}{lb}")
                     for lb in range(NLB)]
                    for hh in range(2)
                ]
                n = 2 * NCK
                i = 0
                for ck in range(NCK):
                    for pm in range(2):
                        for hh in range(2):
                            for lb in range(NLB):
                                nc.tensor.matmul(
                                    pq[hh][lb],
                                    wq_sb[:, ck, 2 * hp + hh, pm, :],
                                    xt[pm][:, ck, lb * 512:(lb + 1) * 512],
                                    start=(i == 0),
                                    stop=(i == n - 1),
                                )
                        i += 1
                for hh in range(2):
                    for lb in range(NLB):
                        nc.vector.tensor_copy(
                            out=qs[:, 2 * hp + hh, lb * 512:(lb + 1) * 512], in_=pq[hh][lb]
                        )

        # ---------- P3: load ct + wk/wv, K and V projections ----------
        exp_pool = ctx.enter_context(tc.tile_pool(name="exp", bufs=2))
        ps_s = ctx.enter_context(tc.tile_pool(name="ps_s", bufs=2, space="PSUM"))

        with (
            tc.tile_pool(name="ct", bufs=1) as ct_pool,
            tc.tile_pool(name="wkv", bufs=1) as wkv_pool,
            tc.tile_pool(name="ps_proj2", bufs=1, space="PSUM") as ps_proj,
            tc.tile_pool(name="ps_v", bufs=2, space="PSUM") as ps_v,
        ):
            wk_sb = wkv_pool.tile([128, NCK, HPC, 2, D2], BF16, tag="wkv")
            wv_sb = wkv_pool.tile([128, NCK, 2, HPC * D2], BF16, tag="wkv", name="wv_sb")
            ct = [ct_pool.tile([128, NCK, S], BF16, tag=f"ct{t}", name=f"ct{t}") for t in range(2)]
            for ck in range(NCK):
                csl = slice(ck * 128, (ck + 1) * 128)
                nc.gpsimd.dma_start(out=wk_sb[:, ck], in_=wk[:, ck])
                nc.sync.dma_start(out=ct[0][:, ck, :], in_=ct_r[csl, :])
                nc.scalar.dma_start(out=ct[1][:, ck, :], in_=ct_i[csl, :])
                nc.gpsimd.dma_start(out=wv_sb[:, ck], in_=wv[:, ck])
            for h in range(HPC):
                for rnd in range(2):
                    pk = [ps_proj.tile([128, 512], F32, tag=f"pk{j}", name=f"pk{j}") for j in range(2)]
                    n = 2 * NCK
                    i = 0
                    for ck in range(NCK):
                        for pm in range(2):
                            for j in range(2):
                                sb = 2 * rnd + j
                                nc.tensor.matmul(
                                    pk[j],
                                    wk_sb[:, ck, h, pm, :],
                                    ct[pm][:, ck, sb * 512:(sb + 1) * 512],
                                    start=(i == 0),
                                    stop=(i == n - 1),
                                )
                            i += 1
                    for j in range(2):
                        sb = 2 * rnd + j
                        nc.vector.tensor_copy(out=ks[:, h, sb * 512:(sb + 1) * 512], in_=pk[j])
            for st in range(NST):
                pv = ps_v.tile([128, 512], F32, tag="pv")
                n = 2 * NCK
                i = 0
                for ck in range(NCK):
                    for pm in range(2):
                        nc.tensor.matmul(
                            pv,
                            ct[pm][:, ck, st * 128:(st + 1) * 128],
                            wv_sb[:, ck, pm, :],
                            start=(i == 0),
                            stop=(i == n - 1),
                        )
                        i += 1
                nc.vector.tensor_copy(out=vs[:, st, :], in_=pv)

        # ---------- P4+P5 fused: attention + output projection, lb-outer ----------
        with (
            tc.tile_pool(name="late", bufs=1) as late_pool,
            tc.tile_pool(name="exp2", bufs=3) as exp_pool2,
            tc.tile_pool(name="otp", bufs=2) as ot_pool,
            tc.tile_pool(name="ysb", bufs=4) as ysb_pool,
            tc.tile_pool(name="ps_d", bufs=1, space="PSUM") as ps_d,
            tc.tile_pool(name="ps_o", bufs=1, space="PSUM") as ps_o,
            tc.tile_pool(name="ps_y", bufs=1, space="PSUM") as ps_y,
        ):
            ones = late_pool.tile([128, D2], EXP_DT)
            nc.vector.memset(ones, 1.0)
            wo_sb = late_pool.tile([128, HPC, 2, NEB, 512], OT_DT, tag="wo", name="wo_sb")
            nc.gpsimd.dma_start(out=wo_sb, in_=wo[:, :, :, :, :])
            expt_tiles = {}

            def scores_exp(lb, h, interleave=None):
                lsl = slice(lb * 512, (lb + 1) * 512)
                pool_h = exp_pool if lb == 0 else exp_pool2
                expt = pool_h.tile([128, NST, 512], EXP_DT, tag="expt", name="expt")
                expt_tiles[(lb, h)] = expt
                for pr in range(NST // 2):
                    pscore = ps_s.tile([128, 2, 512], F32, tag="pscore")
                    for j in range(2):
                        st = 2 * pr + j
                        nc.tensor.matmul(
                            pscore[:, j, :],
                            ks[:, h, st * 128:(st + 1) * 128],
                            qs[:, h, lsl],
                            start=True,
                            stop=True,
                            skip_group_check=True,
                        )
                    nc.scalar.activation(
                        out=expt[:, 2 * pr:2 * pr + 2, :],
                        in_=pscore,
                        func=mybir.ActivationFunctionType.Exp,
                        scale=SCALE,
                    )
                    if interleave is not None:
                        interleave(pr)

            def oproj_group(lb, ot, jt, eb):
                lt = lb * 4 + jt
                lrow = slice(lt * 128, (lt + 1) * 128)
                esl = slice(eb * 512, (eb + 1) * 512)
                py = [ps_y.tile([128, 512], F32, tag=f"py{ri}", name=f"py{ri}")
                      for ri in range(2)]
                for h in range(HPC):
                    for ri in range(2):
                        nc.tensor.matmul(
                            py[ri],
                            ot[:, h, jt * 128:(jt + 1) * 128],
                            wo_sb[:, h, ri, eb, :],
                            start=(h == 0),
                            stop=(h == HPC - 1),
                        )
                yr_t = ysb_pool.tile([128, 512], BF16, tag="yrt")
                nc.vector.tensor_copy(out=yr_t, in_=py[0])
                nc.sync.dma_start(out=y_r[lrow, esl], in_=yr_t)
                yi_t = ysb_pool.tile([128, 512], BF16, tag="yit")
                nc.vector.tensor_copy(out=yi_t, in_=py[1])
                nc.sync.dma_start(out=y_i[lrow, esl], in_=yi_t)

            scores_exp(0, 0)
            for lb in range(NLB):
                ot = ot_pool.tile([128, HPC, 512], OT_DT, tag="ot", name="ot")
                for h in range(HPC):
                    expt = expt_tiles.pop((lb, h))
                    # av: OT[d2, l] accumulated over s-tiles (reads expt first)
                    pav = ps_o.tile([128, 512], F32, tag="pav")
                    for st in range(NST):
                        nc.tensor.matmul(
                            pav,
                            vs[:, st, h * D2:(h + 1) * D2],
                            expt[:, st, :],
                            start=(st == 0),
                            stop=(st == NST - 1),
                        )
                    # out-of-place tree-sum of the 16 s-tiles: runs parallel
                    # with av (expt is only read), strided pairwise levels
                    tr1 = ot_pool.tile([128, 8, 512], EXP_DT, tag="tr1")
                    tr2 = ot_pool.tile([128, 4, 512], EXP_DT, tag="tr2")
                    ev = expt.rearrange("p (a two) f -> p a two f", two=2)
                    for q in range(4):
                        eng = nc.gpsimd if q % 2 == 0 else nc.vector
                        eng.tensor_add(
                            out=tr1[:, 2 * q:2 * q + 2, :],
                            in0=ev[:, 2 * q:2 * q + 2, 0, :],
                            in1=ev[:, 2 * q:2 * q + 2, 1, :],
                        )
                    t1v = tr1.rearrange("p (a two) f -> p a two f", two=2)
                    for q in range(2):
                        nc.vector.tensor_add(
                            out=tr2[:, 2 * q:2 * q + 2, :],
                            in0=t1v[:, 2 * q:2 * q + 2, 0, :],
                            in1=t1v[:, 2 * q:2 * q + 2, 1, :],
                        )
                    t2v = tr2.rearrange("p (a two) f -> p a two f", two=2)
                    nc.vector.tensor_add(
                        out=tr1[:, 0:2, :], in0=t2v[:, :, 0, :], in1=t2v[:, :, 1, :])
                    nc.vector.tensor_add(
                        out=tr2[:, 0, :], in0=tr1[:, 0, :], in1=tr1[:, 1, :])
                    pden = ps_d.tile([128, 512], F32, tag="pden")
                    nc.tensor.matmul(pden, ones, tr2[:, 0, :], start=True, stop=True)
                    recip = ot_pool.tile([128, 512], F32, tag="recip")
                    nc.vector.reciprocal(out=recip, in_=pden)
                    nc.vector.tensor_mul(out=ot[:, h, :], in0=pav, in1=recip)
                    if h + 1 < HPC:
                        scores_exp(lb, h + 1)

                # output projection for this l-block (needs all heads' ot),
                # interleaved with next lb's first-head scores so ACT overlaps
                groups = [(jt, eb) for jt in range(4) for eb in range(NEB)]
                if lb + 1 < NLB:
                    it = iter(groups)

                    def inter(pr, it=it, lb=lb, ot=ot):
                        g = next(it, None)
                        if g is not None:
                            oproj_group(lb, ot, g[0], g[1])

                    scores_exp(lb + 1, 0, interleave=inter)
                    for g in it:
                        oproj_group(lb, ot, g[0], g[1])
                else:
                    for g in groups:
                        oproj_group(lb, ot, g[0], g[1])


def _prep_core_inputs(inputs, core):
    """Slice + host-prepare activation/weight layouts for one core."""
    import ml_dtypes

    BFD = ml_dtypes.bfloat16
    b = core // 4
    g = core % 4
    hcols = slice(g * HPC * D, (g + 1) * HPC * D)  # 256 channel cols/rows

    wq_r = inputs["wq_r"][:, hcols]
    wq_i = inputs["wq_i"][:, hcols]
    wk_r = inputs["wk_r"][:, hcols]
    wk_i = inputs["wk_i"][:, hcols]
    wv_r = inputs["wv_r"][:, hcols]
    wv_i = inputs["wv_i"][:, hcols]
    wo_r = inputs["wo_r"][hcols, :]
    wo_i = inputs["wo_i"][hcols, :]

    def pmajor(w):
        # [C, ...] -> [128, NCK, ...]
        return np.ascontiguousarray(
            w.reshape(NCK, 128, *w.shape[1:]).transpose(
                1, 0, *range(2, w.ndim + 1))
        )

    def stack_lhst(wr, wi):
        # [C, HPC, 2, D2]: pm=0 -> [wr | wi], pm=1 -> [-wi | wr]
        out = np.empty((C, HPC, 2, D2), np.float32)
        for hh in range(HPC):
            cs = slice(hh * D, (hh + 1) * D)
            out[:, hh, 0, :D] = wr[:, cs]
            out[:, hh, 0, D:] = wi[:, cs]
            out[:, hh, 1, :D] = -wi[:, cs]
            out[:, hh, 1, D:] = wr[:, cs]
        return pmajor(out.astype(BFD))

    def stack_rhs_v(wr, wi):
        # [C, 2, HPC*D2]
        out = np.empty((C, 2, HPC * D2), np.float32)
        for hh in range(HPC):
            cs = slice(hh * D, (hh + 1) * D)
            out[:, 0, hh * D2:hh * D2 + D] = wr[:, cs]
            out[:, 0, hh * D2 + D:(hh + 1) * D2] = wi[:, cs]
            out[:, 1, hh * D2:hh * D2 + D] = -wi[:, cs]
            out[:, 1, hh * D2 + D:(hh + 1) * D2] = wr[:, cs]
        return pmajor(out.astype(BFD))

    def stack_wo(wr, wi):
        # [D2, HPC, 2, NEB, 512]; rows 0:D multiply Or, D:D2 multiply Oi
        out = np.empty((D2, HPC, 2, NEB, 512), np.float32)
        for hh in range(HPC):
            rs = slice(hh * D, (hh + 1) * D)
            for eb in range(NEB):
                esl = slice(eb * 512, (eb + 1) * 512)
                out[:D, hh, 0, eb, :] = wr[rs, esl]
                out[D:, hh, 0, eb, :] = -wi[rs, esl]
                out[:D, hh, 1, eb, :] = wi[rs, esl]
                out[D:, hh, 1, eb, :] = wr[rs, esl]
        return np.ascontiguousarray(out.astype(BFD))

    def tcast(x):
        return np.ascontiguousarray(x.T.astype(BFD))

    return {
        "xt_r": tcast(inputs["inputs_real"][b]),
        "xt_i": tcast(inputs["inputs_imag"][b]),
        "ct_r": tcast(inputs["context_real"][b]),
        "ct_i": tcast(inputs["context_imag"][b]),
        "wq": stack_lhst(wq_r, wq_i),
        "wk": stack_lhst(wk_r, wk_i),
        "wv": stack_rhs_v(wv_r, wv_i),
        "wo": stack_wo(wo_r, wo_i),
    }


def get_program():
    if "nc" not in _CACHE:
        _CACHE["nc"] = _build_program()
    return _CACHE["nc"]


def kernel(**inputs):
    nc = get_program()
    in_maps = [_prep_core_inputs(inputs, core) for core in range(8)]
    res = run_bass_kernel_spmd(nc, in_maps, core_ids=list(range(8)))

    yr = np.zeros((B, L, C), np.float32)
    yi = np.zeros((B, L, C), np.float32)
    for core in range(8):
        b = core // 4
        yr[b] += res.results[core]["y_r"].astype(np.float32)
        yi[b] += res.results[core]["y_i"].astype(np.float32)
    yr += inputs["bo_r"][None, None, :]
    yi += inputs["bo_i"][None, None, :]
    return np.stack([yr, yi], axis=0)


# revision 16
# speedup vs baseline: 1.0644x; 1.0157x over previous
"""Trainium2 Bass kernel for nn_ComplexCrossAttention.

Sharding: 8 cores = 2 batches x 4 head-groups (4 heads each).
Each core computes, for its (b, head-group):
  - complex Q/K/V projections (column-sharded by head) in transposed layout
  - attention scoresT = (qr.kr + qi.ki)*scale with s on partitions
  - softmax (no max-subtraction; scores are provably small) via exp + column-sum
  - av in transposed layout -> OT [d2, l]
  - partial output projection (row-sharded by head)
Host sums the 4 partial y per batch (bf16 partials) and adds the bias.

All activations are uploaded HOST-TRANSPOSED in bf16 ([C, L] layout), so the
kernel has no cast or DMA-transpose pipeline; weights are uploaded p-major so
every weight DMA row is >= 2KB contiguous. Outputs return as bf16 partials.
"""

import sys

import numpy as np

try:
    import concourse.bacc as bacc
except ImportError:  # pragma: no cover - fallback for bare environments
    sys.path.insert(0, "/opt/trn_rl_repo")
    import concourse.bacc as bacc

import concourse.mybir as mybir
import concourse.tile as tile
from concourse.bass_utils import run_bass_kernel_spmd

F32 = mybir.dt.float32
BF16 = mybir.dt.bfloat16
F32R = mybir.dt.float32r

# ---- problem constants (hardcoded per contract) ----
B, L, S, C = 2, 2048, 2048, 1024
H, D = 16, 64
SCALE = float(1.0 / np.sqrt(np.float32(D)))
HPC = 4          # heads per core
D2 = 2 * D       # stacked (real|imag) head dim = 128
NCK = C // 128   # contraction chunks = 8
NLB = L // 512   # l-blocks = 4
NST = S // 128   # s-tiles = 16
NLT = L // 128   # l-tiles = 16
NEB = 2          # e-blocks of 512 in C

# ---- dtype configuration ----
QS_DT = BF16     # Qs/Ks (scores operands)
EXP_DT = BF16    # expT / Vs / ones (av + denom operands)
VS_DT = EXP_DT
OT_DT = BF16     # OT / wo (o-proj operands)

_CACHE = {}


def _build_program():
    nc = bacc.Bacc("TRN2", target_bir_lowering=False, debug=False, num_devices=8)

    # per-core external inputs: host-transposed bf16 activations [C, L]
    xt_r = nc.dram_tensor("xt_r", [C, L], BF16, kind="ExternalInput")
    xt_i = nc.dram_tensor("xt_i", [C, L], BF16, kind="ExternalInput")
    ct_r = nc.dram_tensor("ct_r", [C, S], BF16, kind="ExternalInput")
    ct_i = nc.dram_tensor("ct_i", [C, S], BF16, kind="ExternalInput")
    # stacked complex projection weights (host-prepared, bf16, p-major)
    # wq/wk: [128, NCK, HPC, 2, D2]  lhsT tiles
    wq = nc.dram_tensor("wq", [128, NCK, HPC, 2, D2], BF16, kind="ExternalInput")
    wk = nc.dram_tensor("wk", [128, NCK, HPC, 2, D2], BF16, kind="ExternalInput")
    # wv: [128, NCK, 2, HPC*128]  rhs tiles
    wv = nc.dram_tensor("wv", [128, NCK, 2, HPC * D2], BF16, kind="ExternalInput")
    # wo: [128, HPC, 2, NEB, 512]  rhs tiles (p = d2 row)
    wo = nc.dram_tensor("wo", [D2, HPC, 2, NEB, 512], OT_DT, kind="ExternalInput")

    y_r = nc.dram_tensor("y_r", [L, C], BF16, kind="ExternalOutput")
    y_i = nc.dram_tensor("y_i", [L, C], BF16, kind="ExternalOutput")

    with tile.TileContext(nc) as tc:
        _emit(nc, tc, xt_r, xt_i, ct_r, ct_i, wq, wk, wv, wo, y_r, y_i)

    nc.compile()
    return nc


def _emit(nc, tc, xt_r, xt_i, ct_r, ct_i, wq, wk, wv, wo, y_r, y_i):
    from contextlib import ExitStack

    ctx = ExitStack()
    with ctx:
        attn_sb = ctx.enter_context(tc.tile_pool(name="attn_sb", bufs=1))
        wpool = ctx.enter_context(tc.tile_pool(name="wpool", bufs=1))

        # persistent attention operands
        qs = attn_sb.tile([128, HPC, L], QS_DT)            # [d2, h, l]
        ks = attn_sb.tile([128, HPC, S], QS_DT)            # [d2, h, s]
        vs = attn_sb.tile([128, NST, HPC * D2], VS_DT)     # [s-part, st, d2all]

        # ---------- P1+P2: load xt + wq, Q projection ----------
        with (
            tc.tile_pool(name="xt", bufs=1) as xt_pool,
            tc.tile_pool(name="wqk", bufs=1) as wqk_pool,
            tc.tile_pool(name="ps_proj", bufs=1, space="PSUM") as ps_proj,
        ):
            wq_sb = wqk_pool.tile([128, NCK, HPC, 2, D2], BF16, tag="wq")
            wk_sb = wpool.tile([128, NCK, HPC, 2, D2], BF16, tag="wk")
            wv_sb = wpool.tile([128, NCK, 2, HPC * D2], BF16, tag="wv")
            xt = [xt_pool.tile([128, NCK, L], BF16, tag=f"xt{t}", name=f"xt{t}") for t in range(2)]
            for ck in range(NCK):
                csl = slice(ck * 128, (ck + 1) * 128)
                nc.gpsimd.dma_start(out=wq_sb[:, ck], in_=wq[:, ck])
                for q in range(4):
                    qsl = slice(q * 512, (q + 1) * 512)
                    nc.sync.dma_start(out=xt[0][:, ck, qsl], in_=xt_r[csl, qsl])
                    nc.scalar.dma_start(out=xt[1][:, ck, qsl], in_=xt_i[csl, qsl])
            for ck in range(NCK):
                nc.gpsimd.dma_start(out=wk_sb[:, ck], in_=wk[:, ck])
                nc.gpsimd.dma_start(out=wv_sb[:, ck], in_=wv[:, ck])
            for hp in range(HPC // 2):
                pq = [
                    [ps_proj.tile([128, 512], F32, tag=f"pq{hh}{lb}", name=f"pq{hh}{lb}")
                     for lb in range(NLB)]
                    for hh in range(2)
                ]
                n = 2 * NCK
                i = 0
                for ck in range(NCK):
                    for pm in range(2):
                        for hh in range(2):
                            for lb in range(NLB):
                                nc.tensor.matmul(
                                    pq[hh][lb],
                                    wq_sb[:, ck, 2 * hp + hh, pm, :],
                                    xt[pm][:, ck, lb * 512:(lb + 1) * 512],
                                    start=(i == 0),
                                    stop=(i == n - 1),
                                )
                        i += 1
                for hh in range(2):
                    for lb in range(NLB):
                        nc.vector.tensor_copy(
                            out=qs[:, 2 * hp + hh, lb * 512:(lb + 1) * 512], in_=pq[hh][lb]
                        )

        # ---------- P3: load ct + wk/wv, K and V projections ----------
        exp_pool = ctx.enter_context(tc.tile_pool(name="exp", bufs=2))
        ps_s = ctx.enter_context(tc.tile_pool(name="ps_s", bufs=2, space="PSUM"))

        with (
            tc.tile_pool(name="ct", bufs=1) as ct_pool,
            tc.tile_pool(name="wkv", bufs=1) as wkv_pool,
            tc.tile_pool(name="ps_proj2", bufs=1, space="PSUM") as ps_proj,
            tc.tile_pool(name="ps_v", bufs=2, space="PSUM") as ps_v,
        ):
            ct = [ct_pool.tile([128, NCK, S], BF16, tag=f"ct{t}", name=f"ct{t}") for t in range(2)]
            for ck in range(NCK):
                csl = slice(ck * 128, (ck + 1) * 128)
                for q in range(4):
                    qsl = slice(q * 512, (q + 1) * 512)
                    nc.sync.dma_start(out=ct[0][:, ck, qsl], in_=ct_r[csl, qsl])
                    nc.scalar.dma_start(out=ct[1][:, ck, qsl], in_=ct_i[csl, qsl])
            for h in range(HPC):
                for rnd in range(2):
                    pk = [ps_proj.tile([128, 512], F32, tag=f"pk{j}", name=f"pk{j}") for j in range(2)]
                    n = 2 * NCK
                    i = 0
                    for ck in range(NCK):
                        for pm in range(2):
                            for j in range(2):
                                sb = 2 * rnd + j
                                nc.tensor.matmul(
                                    pk[j],
                                    wk_sb[:, ck, h, pm, :],
                                    ct[pm][:, ck, sb * 512:(sb + 1) * 512],
                                    start=(i == 0),
                                    stop=(i == n - 1),
                                )
                            i += 1
                    for j in range(2):
                        sb = 2 * rnd + j
                        nc.vector.tensor_copy(out=ks[:, h, sb * 512:(sb + 1) * 512], in_=pk[j])
            for st in range(NST):
                pv = ps_v.tile([128, 512], F32, tag="pv")
                n = 2 * NCK
                i = 0
                for ck in range(NCK):
                    for pm in range(2):
                        nc.tensor.matmul(
                            pv,
                            ct[pm][:, ck, st * 128:(st + 1) * 128],
                            wv_sb[:, ck, pm, :],
                            start=(i == 0),
                            stop=(i == n - 1),
                        )
                        i += 1
                nc.vector.tensor_copy(out=vs[:, st, :], in_=pv)

        # ---------- P4+P5 fused: attention + output projection, lb-outer ----------
        with (
            tc.tile_pool(name="late", bufs=1) as late_pool,
            tc.tile_pool(name="exp2", bufs=3) as exp_pool2,
            tc.tile_pool(name="otp", bufs=2) as ot_pool,
            tc.tile_pool(name="trp", bufs=1) as tr_pool,
            tc.tile_pool(name="ysb", bufs=3) as ysb_pool,
            tc.tile_pool(name="ps_d", bufs=1, space="PSUM") as ps_d,
            tc.tile_pool(name="ps_o", bufs=1, space="PSUM") as ps_o,
            tc.tile_pool(name="ps_y", bufs=1, space="PSUM") as ps_y,
        ):
            ones = late_pool.tile([128, D2], EXP_DT)
            nc.vector.memset(ones, 1.0)
            wo_sb = late_pool.tile([128, HPC, 2, NEB, 512], OT_DT, tag="wo", name="wo_sb")
            nc.gpsimd.dma_start(out=wo_sb, in_=wo[:, :, :, :, :])
            expt_tiles = {}

            def scores_exp(lb, h, interleave=None):
                lsl = slice(lb * 512, (lb + 1) * 512)
                pool_h = exp_pool if lb == 0 else exp_pool2
                expt = pool_h.tile([128, NST, 512], EXP_DT, tag="expt", name="expt")
                expt_tiles[(lb, h)] = expt
                for pr in range(NST // 2):
                    pscore = ps_s.tile([128, 2, 512], F32, tag="pscore")
                    for j in range(2):
                        st = 2 * pr + j
                        nc.tensor.matmul(
                            pscore[:, j, :],
                            ks[:, h, st * 128:(st + 1) * 128],
                            qs[:, h, lsl],
                            start=True,
                            stop=True,
                            skip_group_check=True,
                        )
                    nc.scalar.activation(
                        out=expt[:, 2 * pr:2 * pr + 2, :],
                        in_=pscore,
                        func=mybir.ActivationFunctionType.Exp,
                        scale=SCALE,
                    )
                    if interleave is not None:
                        interleave(pr)

            def oproj_group(lb, ot, jt, eb):
                lt = lb * 4 + jt
                lrow = slice(lt * 128, (lt + 1) * 128)
                esl = slice(eb * 512, (eb + 1) * 512)
                py = [ps_y.tile([128, 512], F32, tag=f"py{ri}", name=f"py{ri}")
                      for ri in range(2)]
                for h in range(HPC):
                    for ri in range(2):
                        nc.tensor.matmul(
                            py[ri],
                            ot[:, h, jt * 128:(jt + 1) * 128],
                            wo_sb[:, h, ri, eb, :],
                            start=(h == 0),
                            stop=(h == HPC - 1),
                        )
                yr_t = ysb_pool.tile([128, 512], BF16, tag="yrt")
                nc.vector.tensor_copy(out=yr_t, in_=py[0])
                nc.sync.dma_start(out=y_r[lrow, esl], in_=yr_t)
                yi_t = ysb_pool.tile([128, 512], BF16, tag="yit")
                nc.vector.tensor_copy(out=yi_t, in_=py[1])
                nc.sync.dma_start(out=y_i[lrow, esl], in_=yi_t)

            scores_exp(0, 0)
            for lb in range(NLB):
                ot = ot_pool.tile([128, HPC, 512], OT_DT, tag="ot", name="ot")
                for h in range(HPC):
                    expt = expt_tiles.pop((lb, h))
                    # av: OT[d2, l] accumulated over s-tiles (reads expt first)
                    pav = ps_o.tile([128, 512], F32, tag="pav")
                    for st in range(NST):
                        nc.tensor.matmul(
                            pav,
                            vs[:, st, h * D2:(h + 1) * D2],
                            expt[:, st, :],
                            start=(st == 0),
                            stop=(st == NST - 1),
                        )
                    # out-of-place tree-sum of the 16 s-tiles: runs parallel
                    # with av (expt is only read), strided pairwise levels
                    tr1 = tr_pool.tile([128, 8, 512], EXP_DT, tag="tr1")
                    tr2 = tr_pool.tile([128, 4, 512], EXP_DT, tag="tr2")
                    ev = expt.rearrange("p (a two) f -> p a two f", two=2)
                    for q in range(4):
                        eng = nc.gpsimd if q % 2 == 0 else nc.vector
                        eng.tensor_add(
                            out=tr1[:, 2 * q:2 * q + 2, :],
                            in0=ev[:, 2 * q:2 * q + 2, 0, :],
                            in1=ev[:, 2 * q:2 * q + 2, 1, :],
                        )
                    t1v = tr1.rearrange("p (a two) f -> p a two f", two=2)
                    for q in range(2):
                        nc.vector.tensor_add(
                            out=tr2[:, 2 * q:2 * q + 2, :],
                            in0=t1v[:, 2 * q:2 * q + 2, 0, :],
                            in1=t1v[:, 2 * q:2 * q + 2, 1, :],
                        )
                    t2v = tr2.rearrange("p (a two) f -> p a two f", two=2)
                    nc.vector.tensor_add(
                        out=tr1[:, 0:2, :], in0=t2v[:, :, 0, :], in1=t2v[:, :, 1, :])
                    nc.vector.tensor_add(
                        out=tr2[:, 0, :], in0=tr1[:, 0, :], in1=tr1[:, 1, :])
                    pden = ps_d.tile([128, 512], F32, tag="pden")
                    nc.tensor.matmul(pden, ones, tr2[:, 0, :], start=True, stop=True)
                    recip = ot_pool.tile([128, 512], F32, tag="recip")
                    nc.vector.reciprocal(out=recip, in_=pden)
                    nc.vector.tensor_mul(out=ot[:, h, :], in0=pav, in1=recip)
                    if h + 1 < HPC:
                        scores_exp(lb, h + 1)

                # output projection for this l-block (needs all heads' ot),
                # interleaved with next lb's first-head scores so ACT overlaps
                groups = [(jt, eb) for jt in range(4) for eb in range(NEB)]
                if lb + 1 < NLB:
                    it = iter(groups)

                    def inter(pr, it=it, lb=lb, ot=ot):
                        g = next(it, None)
                        if g is not None:
                            oproj_group(lb, ot, g[0], g[1])

                    scores_exp(lb + 1, 0, interleave=inter)
                    for g in it:
                        oproj_group(lb, ot, g[0], g[1])
                else:
                    for g in groups:
                        oproj_group(lb, ot, g[0], g[1])


def _prep_core_inputs(inputs, core):
    """Slice + host-prepare activation/weight layouts for one core."""
    import ml_dtypes

    BFD = ml_dtypes.bfloat16
    b = core // 4
    g = core % 4
    hcols = slice(g * HPC * D, (g + 1) * HPC * D)  # 256 channel cols/rows

    wq_r = inputs["wq_r"][:, hcols]
    wq_i = inputs["wq_i"][:, hcols]
    wk_r = inputs["wk_r"][:, hcols]
    wk_i = inputs["wk_i"][:, hcols]
    wv_r = inputs["wv_r"][:, hcols]
    wv_i = inputs["wv_i"][:, hcols]
    wo_r = inputs["wo_r"][hcols, :]
    wo_i = inputs["wo_i"][hcols, :]

    def pmajor(w):
        # [C, ...] -> [128, NCK, ...]
        return np.ascontiguousarray(
            w.reshape(NCK, 128, *w.shape[1:]).transpose(
                1, 0, *range(2, w.ndim + 1))
        )

    def stack_lhst(wr, wi):
        # [C, HPC, 2, D2]: pm=0 -> [wr | wi], pm=1 -> [-wi | wr]
        out = np.empty((C, HPC, 2, D2), np.float32)
        for hh in range(HPC):
            cs = slice(hh * D, (hh + 1) * D)
            out[:, hh, 0, :D] = wr[:, cs]
            out[:, hh, 0, D:] = wi[:, cs]
            out[:, hh, 1, :D] = -wi[:, cs]
            out[:, hh, 1, D:] = wr[:, cs]
        return pmajor(out.astype(BFD))

    def stack_rhs_v(wr, wi):
        # [C, 2, HPC*D2]
        out = np.empty((C, 2, HPC * D2), np.float32)
        for hh in range(HPC):
            cs = slice(hh * D, (hh + 1) * D)
            out[:, 0, hh * D2:hh * D2 + D] = wr[:, cs]
            out[:, 0, hh * D2 + D:(hh + 1) * D2] = wi[:, cs]
            out[:, 1, hh * D2:hh * D2 + D] = -wi[:, cs]
            out[:, 1, hh * D2 + D:(hh + 1) * D2] = wr[:, cs]
        return pmajor(out.astype(BFD))

    def stack_wo(wr, wi):
        # [D2, HPC, 2, NEB, 512]; rows 0:D multiply Or, D:D2 multiply Oi
        out = np.empty((D2, HPC, 2, NEB, 512), np.float32)
        for hh in range(HPC):
            rs = slice(hh * D, (hh + 1) * D)
            for eb in range(NEB):
                esl = slice(eb * 512, (eb + 1) * 512)
                out[:D, hh, 0, eb, :] = wr[rs, esl]
                out[D:, hh, 0, eb, :] = -wi[rs, esl]
                out[:D, hh, 1, eb, :] = wi[rs, esl]
                out[D:, hh, 1, eb, :] = wr[rs, esl]
        return np.ascontiguousarray(out.astype(BFD))

    def tcast(x):
        return np.ascontiguousarray(x.T.astype(BFD))

    return {
        "xt_r": tcast(inputs["inputs_real"][b]),
        "xt_i": tcast(inputs["inputs_imag"][b]),
        "ct_r": tcast(inputs["context_real"][b]),
        "ct_i": tcast(inputs["context_imag"][b]),
        "wq": stack_lhst(wq_r, wq_i),
        "wk": stack_lhst(wk_r, wk_i),
        "wv": stack_rhs_v(wv_r, wv_i),
        "wo": stack_wo(wo_r, wo_i),
    }


def get_program():
    if "nc" not in _CACHE:
        _CACHE["nc"] = _build_program()
    return _CACHE["nc"]


def kernel(**inputs):
    nc = get_program()
    in_maps = [_prep_core_inputs(inputs, core) for core in range(8)]
    res = run_bass_kernel_spmd(nc, in_maps, core_ids=list(range(8)))

    yr = np.zeros((B, L, C), np.float32)
    yi = np.zeros((B, L, C), np.float32)
    for core in range(8):
        b = core // 4
        yr[b] += res.results[core]["y_r"].astype(np.float32)
        yi[b] += res.results[core]["y_i"].astype(np.float32)
    yr += inputs["bo_r"][None, None, :]
    yi += inputs["bo_i"][None, None, :]
    return np.stack([yr, yi], axis=0)
